# revision 9
# baseline (speedup 1.0000x reference)
"""Peephole-LSTM Trainium2 kernel v2 (per-core program, SPMD over 8 cores).

Same data-parallel contract as v1 (NB=16 rows/core, 4 col-groups), but the
per-step schedule is restructured so the elementwise tail hides under PE
streaming and the xW input GEMM is interleaved into per-step PE idle slots
(no separate serial precompute phase).

Per-group 1280 weight cols are split into half-slices (A = first 128 H of
each 256-wide group slice, B = second 128):

  up cols per group: [pi_A pf_A pi_B pf_B | i_A f_A | i_B f_B | g_A g_B]
  PSUM [128, 2048] f32 (4 banks):
    bank0 [0:512):    peep (pi_A pf_A pi_B pf_B)
    bank1 [512:768):  pre_i_A pre_f_A; [768:1024): pre_g_A pre_g_B
    bank2 [1024:1280): pre_i_B pre_f_B         (+pad)

  Issue order: peep | if_A + xw/tp inject | g_AB + xw inject | if_B + xw/tp
  inject | transposes | one 512-col xW pre-GEMM chunk.  Banks close in a
  staggered order, so tanh/sigmoid/tanh(g) run under later MM blocks; only
  if_B's sigmoid chain is exposed, and the pre-GEMM chunk shadows it.

  cT is kept as two tiles: cTA (even k-chunks = A halves), cTB (odd).
  Main MMs contract k in order [0,2,4,6,1,3,5,7] so next step can start on
  cTA before cTB's transpose lands.
"""

import numpy as np
import ml_dtypes

import concourse.bass as bass
import concourse.bacc as bacc
import concourse.mybir as mybir
import concourse.tile as tile
from concourse.bass import ds

F32 = mybir.dt.float32
BF16 = mybir.dt.bfloat16
I32 = mybir.dt.int32
AF = mybir.ActivationFunctionType
BF = ml_dtypes.bfloat16

B, T_FULL, I_DIM, H = 128, 1024, 512, 1024
NB = 16              # batch rows per core
NG = 4               # column groups
HG = H // NG         # 256
KC = H // 128        # 8 k-chunks
IC = I_DIM // 128    # 4 input chunks
UPW = NG * 5 * HG    # 5120
XWW = 3 * HG         # 768 xw cols per group: [i_A f_A i_B f_B g_A g_B]
PSW = 1536           # psum main width (3 banks)
KORDER = [0, 2, 4, 6, 1, 3, 5, 7]
PRO_JOBS = 12        # pre-GEMM chunks done before the loop (2 row-tiles)

# ----------------------------------------------------------------- host packing


def pack_weights(U, P, W, P_o, bias):
    U4 = U.reshape(H, 4, NG, 2, 128)     # [h, gate(i f g o), g, half, j]
    P2 = P.reshape(H, 2, NG, 2, 128)     # [h, (pi pf), g, half, j]
    up = np.stack(
        [
            P2[:, 0, :, 0], P2[:, 1, :, 0], P2[:, 0, :, 1], P2[:, 1, :, 1],
            U4[:, 0, :, 0], U4[:, 1, :, 0],
            U4[:, 0, :, 1], U4[:, 1, :, 1],
            U4[:, 2, :, 0], U4[:, 2, :, 1],
        ],
        axis=2,
    )  # [H, NG, 10, 128]
    up_pack = np.ascontiguousarray(
        up.reshape(H, NG * 5 * HG).reshape(KC, 128, UPW)).astype(BF)

    W4 = W.reshape(I_DIM, 4, NG, 2, 128)
    w_re = np.stack(
        [W4[:, 0, :, 0], W4[:, 1, :, 0], W4[:, 0, :, 1], W4[:, 1, :, 1],
         W4[:, 2, :, 0], W4[:, 2, :, 1]],
        axis=2,
    )  # [I, NG, 6, 128]
    w_pack = np.ascontiguousarray(
        w_re.reshape(I_DIM, NG * XWW).reshape(IC, 128, NG * XWW)).astype(BF)

    b4 = bias.reshape(4, NG, 2, 128)
    b_re = np.stack(
        [b4[0, :, 0], b4[1, :, 0], b4[0, :, 1], b4[1, :, 1],
         b4[2, :, 0], b4[2, :, 1]],
        axis=1,
    )  # [NG, 6, 128]
    bias_pack = np.ascontiguousarray(b_re.reshape(1, NG * XWW)).astype(BF)

    uo_pack = np.ascontiguousarray(U[:, 3 * H:].reshape(KC, 128, H)).astype(BF)
    po_pack = np.ascontiguousarray(P_o.reshape(KC, 128, H)).astype(BF)
    wo_pack = np.ascontiguousarray(W[:, 3 * H:].reshape(IC, 128, H)).astype(BF)
    biaso_pack = np.ascontiguousarray(bias[3 * H:].reshape(1, H)).astype(BF)
    return dict(up=up_pack, w=w_pack, biasifg=bias_pack, uo=uo_pack,
                po=po_pack, wo=wo_pack, biaso=biaso_pack)


def pack_consts():
    id16 = np.zeros((128, 16), dtype=BF)
    id16f = np.zeros((128, 16), dtype=np.float32)
    for g in range(NG):
        for j in range(16):
            id16[32 * g + j, j] = 1.0
            id16f[32 * g + j, j] = 1.0
    ones1 = np.ones((1, 128), dtype=BF)
    id128 = np.eye(128, dtype=np.float32)
    return dict(id16=id16, id16f=id16f, ones1=ones1, id128=id128)


def pack_core_inputs(x_core, lens_core, t_steps):
    t = t_steps
    x_tb = np.ascontiguousarray(
        x_core[:, :t, :].transpose(1, 0, 2)).reshape(t * NB, I_DIM)
    xT = np.ascontiguousarray(x_tb.T).reshape(IC, 128, t * NB).astype(BF)
    L = np.minimum(lens_core.astype(np.int64), t)
    x_f = np.ascontiguousarray(x_core[np.arange(NB), L - 1, :]).astype(BF)
    gidx = np.zeros((16, 8), np.int32)
    for r in range(NB):
        for g in range(NG):
            gidx[r, g] = int(L[r]) * 128 + 32 * g + r
            gidx[r, 4 + g] = (int(L[r]) - 1) * 128 + 32 * g + r
    return dict(xT=xT, x_f=x_f, gidx=gidx)


# ----------------------------------------------------------------- the program


def _emit_pre_job(nc, cst, job, w_sb, bias_sb, xtp, xwsbp, psprep, dmaq,
                  xw_hist, xT_in, state):
    """One 512-col chunk of the xW pre-GEMM (row-tile m, chunk c)."""
    m, c = divmod(job, 6)
    if c == 0:
        xt = xtp.tile([128, IC * 128], BF16, tag="xt", name=f"xt{m}")
        for ic in range(IC):
            nc.sync.dma_start(
                xt[:, ic * 128: (ic + 1) * 128],
                xT_in[ic, :, ds(m * 128, 128)],
            )
        xw_sb = xwsbp.tile([128, NG * XWW], BF16, tag="xwsb", name=f"xwsb{m}")
        state["xt"] = xt
        state["xw_sb"] = xw_sb
    xt, xw_sb = state["xt"], state["xw_sb"]
    cs = slice(512 * c, 512 * c + 512)
    pp = psprep.tile([128, 512], F32)
    nc.tensor.matmul(pp[:, :], cst["ones1"][0:1, :], bias_sb[0:1, cs],
                     start=True, stop=False)
    for ic in range(IC):
        nc.tensor.matmul(
            pp[:, :],
            xt[:, ic * 128: (ic + 1) * 128],
            w_sb[:, ic * (NG * XWW) + 512 * c: ic * (NG * XWW) + 512 * c + 512],
            start=False, stop=(ic == IC - 1))
    nc.vector.tensor_copy(xw_sb[:, cs], pp[:])
    if c == 5:
        for g in range(NG):
            nc.sync.dma_start(
                xw_hist[ds(m * 8 * 128, 8 * 128), :].rearrange(
                    "(s gq) j -> s gq j", gq=128)[:, 32 * g: 32 * g + NB, :],
                xw_sb[:, XWW * g: XWW * (g + 1)],
            )


def _phase_loop(nc, tc, cst, t, up_in, w_in, biasifg_in, xT_in, xw_hist,
                c_hist):
    id16 = cst["id16"]
    njobs = 6 * ((t * NB) // 128)
    with (
        tc.tile_pool(name="upw", bufs=1) as upp,
        tc.tile_pool(name="wre", bufs=1) as wrep,
        tc.tile_pool(name="biassb", bufs=1) as biasp,
        tc.tile_pool(name="xt", bufs=2) as xtp,
        tc.tile_pool(name="xwsb", bufs=2) as xwsbp,
        tc.tile_pool(name="xwblk", bufs=4) as xwblkp,
        tc.tile_pool(name="state", bufs=1) as statep,
        tc.tile_pool(name="eltw", bufs=2) as eltp,
        tc.tile_pool(name="psmain", bufs=2, space="PSUM") as psmp,
        tc.tile_pool(name="pspre", bufs=2, space="PSUM") as psprep,
    ):
        up_sb = []
        for k in range(KC):
            u = upp.tile([128, UPW], BF16, tag=f"up{k}", name=f"up{k}")
            nc.sync.dma_start(u[:], up_in[k, :, :])
            up_sb.append(u)
        w_sb = wrep.tile([128, IC * NG * XWW], BF16)
        for ic in range(IC):
            nc.sync.dma_start(
                w_sb[:, ic * NG * XWW: (ic + 1) * NG * XWW], w_in[ic, :, :])
        bias_sb = biasp.tile([1, NG * XWW], BF16)
        nc.sync.dma_start(bias_sb[:], biasifg_in[:])

        c_tiles = [statep.tile([128, HG], F32, tag="cA", name="cA"),
                   statep.tile([128, HG], F32, tag="cB", name="cB")]
        cTA_tiles = [statep.tile([128, 128], BF16, tag="cTA0", name="cTA0"),
                     statep.tile([128, 128], BF16, tag="cTA1", name="cTA1")]
        cTB_tiles = [statep.tile([128, 128], BF16, tag="cTB0", name="cTB0"),
                     statep.tile([128, 128], BF16, tag="cTB1", name="cTB1")]
        nc.vector.memset(c_tiles[0][:], 0.0)
        nc.vector.memset(cTA_tiles[0][:], 0.0)
        nc.vector.memset(cTB_tiles[0][:], 0.0)
        nc.sync.dma_start(c_hist[ds(0, 128), :], c_tiles[0][:])

        pre_state = {}
        for job in range(min(PRO_JOBS, njobs)):
            _emit_pre_job(nc, cst, job, w_sb, bias_sb, xtp, xwsbp, psprep,
                          None, xw_hist, xT_in, pre_state)

        GW = UPW // NG  # 1280 up cols per group

        def main_block(ps, upcol, pscol, width, cTA, cTB, start, stop):
            for ki, k in enumerate(KORDER):
                src = cTA if (k % 2 == 0) else cTB
                scol = 32 * (k // 2)
                for g in range(NG):
                    nc.tensor.matmul(
                        ps[32 * g: 32 * g + NB, pscol: pscol + width],
                        src[:, scol: scol + 16],
                        up_sb[k][:, GW * g + upcol: GW * g + upcol + width],
                        start=start and (ki == 0),
                        stop=stop and (ki == KC - 1),
                        tile_position=(0, 32 * g),
                    )

        def inject(ps, pscol, mv, mvcol, width, stop):
            for g in range(NG):
                sl = slice(32 * g, 32 * g + NB)
                nc.tensor.matmul(
                    ps[sl, pscol: pscol + width], id16[sl, :],
                    mv[sl, mvcol: mvcol + width],
                    start=False, stop=stop, tile_position=(32 * g, 32 * g),
                    skip_group_check=True)

        for step in range(t):
            c_cur = c_tiles[step % 2]
            c_nxt = c_tiles[(step + 1) % 2]
            cTA_cur = cTA_tiles[step % 2]
            cTA_nxt = cTA_tiles[(step + 1) % 2]
            cTB_cur = cTB_tiles[step % 2]
            cTB_nxt = cTB_tiles[(step + 1) % 2]

            xws = xwblkp.tile([128, XWW], BF16, tag="xws", name="xws")
            nc.sync.dma_start(xws[:], xw_hist[ds(step * 128, 128), :])

            ps = psmp.tile([128, PSW], F32)

            # bank0: peep (pi_A pf_A pi_B pf_B)
            main_block(ps, 0, 0, 512, cTA_cur, cTB_cur, True, True)
            # tanh of peep halves -> tp bf16
            tpA = eltp.tile([128, 256], BF16, tag="tpA", name="tpA")
            nc.scalar.activation(tpA[:], ps[:, 0:256], AF.Tanh)
            tpB = eltp.tile([128, 256], BF16, tag="tpB", name="tpB")
            nc.scalar.activation(tpB[:], ps[:, 256:512], AF.Tanh)

            # bank1: if_A.  k0(start) -> xw inject -> k1..7 -> tp inject(stop)
            for ki, k in enumerate(KORDER):
                src = cTA_cur if (k % 2 == 0) else cTB_cur
                scol = 32 * (k // 2)
                for g in range(NG):
                    nc.tensor.matmul(
                        ps[32 * g: 32 * g + NB, 512:768],
                        src[:, scol: scol + 16],
                        up_sb[k][:, GW * g + 512: GW * g + 768],
                        start=(ki == 0), stop=False,
                        tile_position=(0, 32 * g),
                    )
                if ki == 0:
                    inject(ps, 512, xws, 0, 256, False)
            inject(ps, 512, tpA, 0, 256, True)
            sigA = eltp.tile([128, 256], F32, tag="sigA", name="sigA")
            nc.scalar.activation(sigA[:], ps[:, 512:768], AF.Sigmoid)

            # bank3: g_AB.  k0(start) -> xw inject -> k1..7(stop)
            for ki, k in enumerate(KORDER):
                src = cTA_cur if (k % 2 == 0) else cTB_cur
                scol = 32 * (k // 2)
                for g in range(NG):
                    nc.tensor.matmul(
                        ps[32 * g: 32 * g + NB, 768:1024],
                        src[:, scol: scol + 16],
                        up_sb[k][:, GW * g + 1024: GW * g + 1280],
                        start=(ki == 0), stop=(ki == KC - 1),
                        tile_position=(0, 32 * g),
                    )
                if ki == 0:
                    inject(ps, 768, xws, 512, 256, False)
            tg = eltp.tile([128, 256], F32, tag="tg", name="tg")
            nc.scalar.activation(tg[:], ps[:, 768:1024], AF.Tanh)

            # A-half elementwise (hidden under if_B MMs)
            t2A = eltp.tile([128, 128], F32, tag="t2A", name="t2A")
            nc.vector.tensor_mul(t2A[:], sigA[:, 128:256], c_cur[:, 0:128])
            t1A = eltp.tile([128, 128], F32, tag="t1A", name="t1A")
            nc.vector.tensor_mul(t1A[:], sigA[:, 0:128], tg[:, 0:128])
            nc.vector.tensor_add(c_nxt[:, 0:128], t1A[:], t2A[:])

            # bank2: if_B.  k0(start) -> xw inject -> k1..7 -> tp inject(stop)
            for ki, k in enumerate(KORDER):
                src = cTA_cur if (k % 2 == 0) else cTB_cur
                scol = 32 * (k // 2)
                for g in range(NG):
                    nc.tensor.matmul(
                        ps[32 * g: 32 * g + NB, 1024:1280],
                        src[:, scol: scol + 16],
                        up_sb[k][:, GW * g + 768: GW * g + 1024],
                        start=(ki == 0), stop=False,
                        tile_position=(0, 32 * g),
                    )
                if ki == 0:
                    inject(ps, 1024, xws, 256, 256, False)
            inject(ps, 1024, tpB, 0, 256, True)

            # A-half cT via xbar DMA transpose (off the PE/DVE critical path)
            cbfA = eltp.tile([128, 128], BF16, tag="cbfA", name="cbfA")
            nc.vector.tensor_copy(cbfA[:], c_nxt[:, 0:128])
            nc.sync.dma_start_transpose(cTA_nxt[:], cbfA[:])

            sigB = eltp.tile([128, 256], F32, tag="sigB", name="sigB")
            nc.scalar.activation(sigB[:], ps[:, 1024:1280], AF.Sigmoid)
            t2B = eltp.tile([128, 128], F32, tag="t2B", name="t2B")
            nc.vector.tensor_mul(t2B[:], sigB[:, 128:256], c_cur[:, 128:256])
            t1B = eltp.tile([128, 128], F32, tag="t1B", name="t1B")
            nc.vector.tensor_mul(t1B[:], sigB[:, 0:128], tg[:, 128:256])
            nc.vector.tensor_add(c_nxt[:, 128:256], t1B[:], t2B[:])

            cbfB = eltp.tile([128, 128], BF16, tag="cbfB", name="cbfB")
            nc.vector.tensor_copy(cbfB[:], c_nxt[:, 128:256])
            nc.sync.dma_start_transpose(cTB_nxt[:], cbfB[:])
            nc.sync.dma_start(c_hist[ds((step + 1) * 128, 128), :], c_nxt[:])

            # shadow work: one xW pre-GEMM chunk
            job = PRO_JOBS + step
            if job < njobs:
                _emit_pre_job(nc, cst, job, w_sb, bias_sb, xtp, xwsbp,
                              psprep, None, xw_hist, xT_in, pre_state)


def _phase_finalize(nc, tc, cst, uo_in, po_in, wo_in, biaso_in,
                    gidx_in, xf_in, c_hist, h_out):
    id16, id16f, ones1 = cst["id16"], cst["id16f"], cst["ones1"]
    with (
        tc.tile_pool(name="finw", bufs=1) as finwp,
        tc.tile_pool(name="fin", bufs=1) as finp,
        tc.tile_pool(name="psfin", bufs=1, space="PSUM") as psfp,
        tc.tile_pool(name="psfin2", bufs=1, space="PSUM") as psf2p,
        tc.tile_pool(name="pstf", bufs=1, space="PSUM") as pstfp,
    ):
        uo_sb = finwp.tile([128, KC * H], BF16, tag="uo", name="uo")
        po_sb = finwp.tile([128, KC * H], BF16, tag="po", name="po")
        wo_sb = finwp.tile([128, IC * H], BF16, tag="wo", name="wo")
        for k in range(KC):
            nc.sync.dma_start(uo_sb[:, k * H: (k + 1) * H], uo_in[k, :, :])
            nc.sync.dma_start(po_sb[:, k * H: (k + 1) * H], po_in[k, :, :])
        for c in range(IC):
            nc.sync.dma_start(wo_sb[:, c * H: (c + 1) * H], wo_in[c, :, :])
        bo_sb = finp.tile([1, H], BF16, tag="bo", name="bo")
        nc.sync.dma_start(bo_sb[:], biaso_in[:])
        gidx = finp.tile([16, 8], I32, tag="gidx", name="gidx")
        nc.sync.dma_start(gidx[:], gidx_in[:])
        xf_sb = finp.tile([NB, I_DIM], BF16, tag="xf", name="xf")
        nc.sync.dma_start(xf_sb[:], xf_in[:])

        cout_b = finp.tile([NB, H], F32, tag="cout", name="cout")
        cin_b = finp.tile([NB, H], F32, tag="cin", name="cin")
        for g in range(NG):
            nc.gpsimd.indirect_dma_start(
                out=cout_b[:, HG * g: HG * (g + 1)], out_offset=None,
                in_=c_hist[:],
                in_offset=bass.IndirectOffsetOnAxis(ap=gidx[:, g: g + 1], axis=0),
            )
            nc.gpsimd.indirect_dma_start(
                out=cin_b[:, HG * g: HG * (g + 1)], out_offset=None,
                in_=c_hist[:],
                in_offset=bass.IndirectOffsetOnAxis(ap=gidx[:, 4 + g: 5 + g], axis=0),
            )

        def transpose_to_bf16(src_b, nm, nchunk, ident, psum_dtype):
            dst = finp.tile([128, 16 * nchunk], BF16, tag=nm, name=nm)
            pstf = pstfp.tile([128, 128], psum_dtype, tag="pstf" + nm,
                              name="pstf" + nm)
            for k in range(nchunk):
                nc.tensor.transpose(
                    pstf[:, 16 * k: 16 * k + 16],
                    src_b[0:NB, 128 * k: 128 * k + 128],
                    ident[0:16, 0:16],
                )
            nc.vector.tensor_copy(dst[:], pstf[:, 0: 16 * nchunk])
            return dst

        xtT = transpose_to_bf16(xf_sb, "xtT", IC, id16, BF16)
        cinT = transpose_to_bf16(cin_b, "cinT", KC, id16f, F32)
        coutT = transpose_to_bf16(cout_b, "coutT", KC, id16f, F32)

        ps_o = psfp.tile([NB, H], F32)
        ps_po = psf2p.tile([NB, H], F32)
        for half in range(2):
            cs = slice(512 * half, 512 * half + 512)
            nc.tensor.matmul(ps_o[:, cs], ones1[0:1, 0:NB], bo_sb[0:1, cs],
                             start=True, stop=False)
            for c in range(IC):
                nc.tensor.matmul(
                    ps_o[:, cs], xtT[:, 16 * c: 16 * c + 16],
                    wo_sb[:, c * H + 512 * half: c * H + 512 * half + 512],
                    start=False, stop=False)
            for k in range(KC):
                nc.tensor.matmul(
                    ps_o[:, cs], cinT[:, 16 * k: 16 * k + 16],
                    uo_sb[:, k * H + 512 * half: k * H + 512 * half + 512],
                    start=False, stop=False)
                nc.tensor.matmul(
                    ps_po[:, cs], coutT[:, 16 * k: 16 * k + 16],
                    po_sb[:, k * H + 512 * half: k * H + 512 * half + 512],
                    start=(k == 0), stop=(k == KC - 1))
        tpo = finp.tile([NB, H], BF16, tag="tpo", name="tpo")
        nc.scalar.activation(tpo[:], ps_po[:], AF.Tanh)
        for half in range(2):
            cs = slice(512 * half, 512 * half + 512)
            nc.tensor.matmul(ps_o[:, cs], id16[0:NB, :], tpo[:, cs],
                             start=False, stop=True, skip_group_check=True)
        o_sb = finp.tile([NB, H], F32, tag="osb", name="osb")
        nc.scalar.activation(o_sb[:], ps_o[:], AF.Sigmoid)
        tanc = finp.tile([NB, H], F32, tag="tanc", name="tanc")
        nc.scalar.activation(tanc[:], cout_b[:], AF.Tanh)
        h_sb = finp.tile([NB, H], F32, tag="hsb", name="hsb")
        nc.vector.tensor_mul(h_sb[:], o_sb[:], tanc[:])
        nc.sync.dma_start(h_out[:], h_sb[:])


def build_program(t_steps=T_FULL):
    t = t_steps
    assert (t * NB) % 128 == 0

    nc = bacc.Bacc(None, target_bir_lowering=False, debug=False)
    dp = nc.declare_dram_parameter
    up_in = dp("up", [KC, 128, UPW], BF16, isOutput=False)
    w_in = dp("w", [IC, 128, NG * XWW], BF16, isOutput=False)
    biasifg_in = dp("biasifg", [1, NG * XWW], BF16, isOutput=False)
    uo_in = dp("uo", [KC, 128, H], BF16, isOutput=False)
    po_in = dp("po", [KC, 128, H], BF16, isOutput=False)
    wo_in = dp("wo", [IC, 128, H], BF16, isOutput=False)
    biaso_in = dp("biaso", [1, H], BF16, isOutput=False)
    xT_in = dp("xT", [IC, 128, t * NB], BF16, isOutput=False)
    xf_in = dp("x_f", [NB, I_DIM], BF16, isOutput=False)
    gidx_in = dp("gidx", [16, 8], I32, isOutput=False)
    id16_in = dp("id16", [128, 16], BF16, isOutput=False)
    id16f_in = dp("id16f", [128, 16], F32, isOutput=False)
    id128_in = dp("id128", [128, 128], F32, isOutput=False)
    ones1_in = dp("ones1", [1, 128], BF16, isOutput=False)
    h_out = dp("h_out", [NB, H], F32, isOutput=True)

    xw_hist = nc.dram_tensor("xw_hist", [t * 128, XWW], BF16)
    c_hist = nc.dram_tensor("c_hist", [(t + 1) * 128, HG], F32)

    with tile.TileContext(nc) as tc:
        with tc.tile_pool(name="const", bufs=1) as constp:
            id16 = constp.tile([128, 16], BF16)
            nc.sync.dma_start(id16[:], id16_in[:])
            id16f = constp.tile([128, 16], F32)
            nc.sync.dma_start(id16f[:], id16f_in[:])
            ones1 = constp.tile([1, 128], BF16)
            nc.sync.dma_start(ones1[:], ones1_in[:])
            id128 = constp.tile([128, 128], F32)
            nc.sync.dma_start(id128[:], id128_in[:])
            cst = dict(id16=id16, id16f=id16f, ones1=ones1, id128=id128)

            _phase_loop(nc, tc, cst, t, up_in, w_in, biasifg_in, xT_in,
                        xw_hist, c_hist)
            _phase_finalize(nc, tc, cst, uo_in, po_in, wo_in, biaso_in,
                            gidx_in, xf_in, c_hist, h_out)

    nc.compile()
    return nc


# ------------------------------------------------------- full host-side kernel


def make_in_maps(inputs, t_steps):
    x = np.asarray(inputs["x"], np.float32)
    lens = np.asarray(inputs["lens"]).astype(np.int64)
    wp = pack_weights(np.asarray(inputs["U"], np.float32),
                      np.asarray(inputs["P"], np.float32),
                      np.asarray(inputs["W"], np.float32),
                      np.asarray(inputs["P_o"], np.float32),
                      np.asarray(inputs["bias"], np.float32))
    cp = pack_consts()
    shared = {**wp, **cp}
    in_maps = []
    for core in range(8):
        sl = slice(core * NB, (core + 1) * NB)
        ci = pack_core_inputs(x[sl], lens[sl], t_steps)
        in_maps.append({**shared, **ci})
    return in_maps


def run(inputs, t_steps=T_FULL, trace=False, nc_cache={}):
    from concourse.bass_utils import run_bass_kernel_spmd

    in_maps = make_in_maps(inputs, t_steps)
    if t_steps not in nc_cache:
        nc_cache[t_steps] = build_program(t_steps)
    nc = nc_cache[t_steps]

    kw = {"tmpdir": "/tmp/bass_trace"} if trace else {}
    if trace:
        import os as _os
        _os.makedirs("/tmp/bass_trace", exist_ok=True)
    res = run_bass_kernel_spmd(nc, in_maps, list(range(8)), trace=trace, **kw)
    h = np.concatenate([res.results[i]["h_out"] for i in range(8)], axis=0)
    return h.astype(np.float32), res


LAST_EXEC_NS = None


def kernel(**inputs):
    """Peephole-LSTM forward; returns h at t=lens-1 for each row: [B, H] f32."""
    global LAST_EXEC_NS
    import os
    trace = bool(os.environ.get("BASS_TRACE"))
    h, res = run(inputs, t_steps=T_FULL, trace=trace)
    if res.exec_time_ns is not None:
        LAST_EXEC_NS = res.exec_time_ns
    return h
